# revision 25
# baseline (speedup 1.0000x reference)
"""Trainium2 Bass kernel for nn_Interactor (attention-augmented LSTM).

Problem: B=64, T=512, L=48, DV=DS=H=512.
  per step t: proj_V = x_t W_V^T; proj_R = h W_R^T
              e = tanh(proj_R[:,None,:] + proj_S + proj_V[:,None,:])
              alpha = softmax(e @ w, axis=L); h_ts = alpha @ h_s
              gates = [x_t, h_ts] W_ih^T + h W_hh^T + b; LSTM cell update.

Sharding: data-parallel over batch: 8 cores x 8 batch rows, weights replicated.

Per-core layout: feature dims on SBUF partitions, batch rows on the free dim.
x_t-dependent projections (PV = W_V x_t, GV = W_ihV x_t + b) are precomputed
with large matmuls into DRAM and streamed back per T-chunk.

The recurrent step is one serial dependency chain (wall time = T x chain
latency), so the design minimizes serial stages and cross-engine handoffs:
 - PV[t] is preloaded into the proj_R PSUM accumulator with an identity
   matmul; the e-add reads that PSUM directly with a stride-0 broadcast
   access pattern (no separate rvt stage). The e-add is split between
   VectorE (kc 0-1) and GpSimd/Pool (kc 2-3, from a small SBUF copy since
   Pool cannot read PSUM) into two separate tiles so the two tanh calls
   have independent dependencies.
 - All sigmoids are computed as tanh: host pre-scales i/f/o gate rows of
   W_ih/W_hh/b by 0.5, so ONE tanh over the gate tile yields tanh(i/2),
   tanh(f/2), tanh(g), tanh(o/2) (single activation-table set, no per-step
   reloads). The cell carries C = 2c and Hc = 2h; with fused
   scalar_tensor_tensor ops the cell update is 3 serial DVE stages:
     z1 = (tf+1)*C; z2 = (ti+1)*tg; C' = 0.5*z1 + z2
     tc = tanh(0.5*C'); Hc = (to+1)*tc; h_out = 0.5*Hc.
   (W_R/W_hh columns are pre-scaled by 0.5 to absorb Hc = 2h.)
 - beta = e @ w is computed transposed into a [48, B] per-batch-column
   PSUM layout (e stationary, w moving), so softmax needs NO masks:
   exp(beta) columns feed the h_ts matmuls directly (one [48,128] x
   [48,1] matmul per (d-chunk, b)), the denominator is one ones-column
   matmul, and 1/denom is broadcast across partitions with one more
   matmul; h_ts is normalized by a single tensor_tensor multiply.
 - gates accumulate in two PSUM passes: ident@GV + W_hh @ Hc early
   (overlapped with the attention front, copied to SBUF), then
   ident@that + W_ihS @ h_ts late, with tanh reading PSUM directly.
   All PSUM accumulation groups are strictly sequential per 2KB bank.
"""

import numpy as np

import concourse.bass as bass
import concourse.mybir as mybir
import concourse.bass_isa as bass_isa
import concourse.tile as tile
from concourse import bacc
from concourse.bass_utils import run_bass_kernel_spmd

F32 = mybir.dt.float32
AF = mybir.ActivationFunctionType
ALU = mybir.AluOpType
AX = mybir.AxisListType

B, T_FULL, L = 64, 512, 48
DV, DS, H = 512, 512, 512
G4 = 4 * H
NCORES = 8
BLOC = B // NCORES  # 8
BL = BLOC * L       # 384
KH = H // 128       # 4 H-chunks
KM = G4 // 128      # 16 gate-row chunks
GW = KH * BLOC      # 32


def build_nc(T=T_FULL, Tc=16):
    """Build the per-core Bass program (SPMD; same program all cores)."""
    assert T % Tc == 0
    nc = bacc.Bacc()

    # ---- DRAM I/O (per-core slices fed via in_maps) ----
    hvT = nc.declare_dram_parameter("hvT", [DV, T * BLOC], F32, isOutput=False)
    hsT = nc.declare_dram_parameter("hsT", [DS, BL], F32, isOutput=False)
    hsblB = nc.declare_dram_parameter("hsblB", [BLOC, L, DS], F32, isOutput=False)
    WS_T = nc.declare_dram_parameter("WS_T", [DS, H], F32, isOutput=False)
    WV_T = nc.declare_dram_parameter("WV_T", [DV, H], F32, isOutput=False)
    WihV_T = nc.declare_dram_parameter("WihV_T", [DV, G4], F32, isOutput=False)
    WihS_T = nc.declare_dram_parameter("WihS_T", [DS, G4], F32, isOutput=False)
    Whh_T = nc.declare_dram_parameter("Whh_T", [H, G4], F32, isOutput=False)
    WR_T = nc.declare_dram_parameter("WR_T", [H, H], F32, isOutput=False)
    wvec = nc.declare_dram_parameter("wvec", [H, 1], F32, isOutput=False)
    biasRSV = nc.declare_dram_parameter("biasRSV", [128, KH], F32, isOutput=False)
    biasIH = nc.declare_dram_parameter("biasIH", [128, KM], F32, isOutput=False)
    bw128 = nc.declare_dram_parameter("bw128", [128, 1], F32, isOutput=False)
    ident = nc.declare_dram_parameter("ident", [128, 128], F32, isOutput=False)
    out_c = nc.declare_dram_parameter("out_c", [T, KH, 128, BLOC], F32, isOutput=True)

    # ---- internal DRAM for precomputed projections ----
    GV_d = nc.dram_tensor("GV_d", [T, KM, 128, BLOC], F32)
    PV_d = nc.dram_tensor("PV_d", [T, KH, 128, BLOC], F32)

    NT = T * BLOC  # hvT free size
    NCW = min(512, NT)  # precompute N-chunk width
    n_nc = NT // NCW

    with tile.TileContext(nc) as tc:
        with (
            tc.tile_pool(name="res", bufs=1) as res,       # resident weights etc
            tc.tile_pool(name="state", bufs=2) as state,   # c/h state ping-pong
            tc.tile_pool(name="hout", bufs=2) as houtp,    # h ring (8 steps)
            tc.tile_pool(name="stream", bufs=2) as stream, # GV/PV chunks
            tc.tile_pool(name="work", bufs=2) as work,     # per-step tiles
        ):
            # ---------- resident loads ----------
            wr_sb = res.tile([128, KH, H], F32, tag="wr")
            nc.sync.dma_start(out=wr_sb, in_=WR_T.rearrange("(k p) m -> p k m", p=128))
            wihs_sb = res.tile([128, KH, G4], F32, tag="wihs")
            nc.sync.dma_start(out=wihs_sb, in_=WihS_T.rearrange("(k p) m -> p k m", p=128))
            whh_sb = res.tile([128, KH, G4], F32, tag="whh")
            nc.sync.dma_start(out=whh_sb, in_=Whh_T.rearrange("(k p) m -> p k m", p=128))
            wvec_sb = res.tile([128, KH], F32, tag="wvec")
            nc.sync.dma_start(out=wvec_sb, in_=wvec.rearrange("(k p) o -> p (k o)", p=128))
            bw_sb = res.tile([128, 1], F32, tag="bw")
            nc.sync.dma_start(out=bw_sb, in_=bw128[:, :])
            ident_sb = res.tile([128, 128], F32, tag="ident")
            nc.sync.dma_start(out=ident_sb, in_=ident[:, :])
            ones48_sb = res.tile([L, 1], F32, tag="ones48")
            nc.vector.memset(ones48_sb, 1.0)
            ones1_sb = res.tile([1, 128], F32, tag="ones1")
            nc.vector.memset(ones1_sb, 1.0)
            # h_s in [l, b, d] layout: per (b): [48, 512] for the h_ts matmuls
            hsbl_sb = res.tile([L, BLOC, DS], F32, tag="hsbl")
            nc.sync.dma_start(out=hsbl_sb, in_=hsblB.rearrange("b p d -> p b d"))
            # PS: proj_S + (b_S+b_R+b_V), [128, KH, (b,l)]
            ps_sb = res.tile([128, KH, BL], F32, tag="ps")

            # ---------- precompute phase ----------
            with (
                tc.tile_pool(name="prew", bufs=1) as prew,
                tc.tile_pool(name="prehv", bufs=4) as prehv,
                tc.tile_pool(name="prestg", bufs=2) as prestg,
                tc.tile_pool(name="prepsum", bufs=4, space="PSUM") as prepsum,
            ):
                hs_sb = prew.tile([128, KH, BL], F32, tag="hs")
                nc.sync.dma_start(out=hs_sb, in_=hsT.rearrange("(k p) n -> p k n", p=128))
                brsv_sb = prew.tile([128, KH], F32, tag="brsv")
                nc.sync.dma_start(out=brsv_sb, in_=biasRSV[:, :])
                bih_sb = prew.tile([128, KM], F32, tag="bih")
                nc.sync.dma_start(out=bih_sb, in_=biasIH[:, :])
                ws_sb = prew.tile([128, KH, H], F32, tag="ws")
                nc.sync.dma_start(out=ws_sb, in_=WS_T.rearrange("(k p) m -> p k m", p=128))
                wv_sb = prew.tile([128, KH, H], F32, tag="wv")
                nc.sync.dma_start(out=wv_sb, in_=WV_T.rearrange("(k p) m -> p k m", p=128))
                wihv_sb = prew.tile([128, KH, G4], F32, tag="wihv")
                nc.sync.dma_start(out=wihv_sb, in_=WihV_T.rearrange("(k p) m -> p k m", p=128))

                # PS = W_S @ hsT + biasRSV
                for m in range(KH):
                    pps = prepsum.tile([128, 512], F32, tag="pp")
                    for kc in range(KH):
                        nc.tensor.matmul(
                            pps[:, :BL],
                            ws_sb[:, kc, m * 128:(m + 1) * 128],
                            hs_sb[:, kc, :],
                            start=(kc == 0), stop=(kc == KH - 1),
                        )
                    nc.vector.tensor_scalar_add(ps_sb[:, m, :], pps[:, :BL], brsv_sb[:, m:m + 1])

                # PV / GV over hvT N-chunks of 512
                for ncnk in range(n_nc):
                    nsl = slice(ncnk * NCW, (ncnk + 1) * NCW)
                    hv_t = []
                    for kc in range(KH):
                        t_ = prehv.tile([128, NCW], F32, tag="hv")
                        nc.sync.dma_start(
                            out=t_, in_=hvT[kc * 128:(kc + 1) * 128, nsl])
                        hv_t.append(t_)
                    for m in range(KH):
                        ppv = prepsum.tile([128, NCW], F32, tag="pp")
                        for kc in range(KH):
                            nc.tensor.matmul(
                                ppv, wv_sb[:, kc, m * 128:(m + 1) * 128],
                                hv_t[kc], start=(kc == 0), stop=(kc == KH - 1))
                        stg = prestg.tile([128, NCW], F32, tag="pvstg")
                        nc.scalar.copy(stg, ppv)
                        t0 = ncnk * NCW // BLOC
                        tw = NCW // BLOC
                        nc.sync.dma_start(
                            out=PV_d[t0:t0 + tw, m, :, :].rearrange("t p b -> p t b"),
                            in_=stg.rearrange("p (t b) -> p t b", b=BLOC))
                    for m in range(KM):
                        pgv = prepsum.tile([128, NCW], F32, tag="pp")
                        for kc in range(KH):
                            nc.tensor.matmul(
                                pgv, wihv_sb[:, kc, m * 128:(m + 1) * 128],
                                hv_t[kc], start=(kc == 0), stop=(kc == KH - 1))
                        stg = prestg.tile([128, NCW], F32, tag="gvstg")
                        nc.vector.tensor_scalar_add(stg, pgv, bih_sb[:, m:m + 1])
                        t0 = ncnk * NCW // BLOC
                        tw = NCW // BLOC
                        nc.sync.dma_start(
                            out=GV_d[t0:t0 + tw, m, :, :].rearrange("t p b -> p t b"),
                            in_=stg.rearrange("p (t b) -> p t b", b=BLOC))

            # ---------- recurrence ----------
            psum = tc.alloc_tile_pool(name="psum", bufs=2, space="PSUM")
            czero = res.tile([128, GW], F32, tag="c0z")
            nc.vector.memset(czero, 0.0)
            hzero = res.tile([128, GW], F32, tag="h0z")
            nc.vector.memset(hzero, 0.0)
            c_prev = czero  # C = 2c, [128, (kc,b)]
            h_prev = hzero  # Hc = 2h

            HB = 8  # h ring steps per DMA
            gv_cur = pv_cur = None
            hbuf = None

            for t in range(T):
                ic = t % Tc
                if ic == 0:
                    gv_cur = stream.tile([128, Tc, KM, BLOC], F32, tag="gv")
                    nc.sync.dma_start(
                        out=gv_cur,
                        in_=GV_d[t:t + Tc].rearrange("t m p b -> p t m b"))
                    pv_cur = stream.tile([128, Tc, KH, BLOC], F32, tag="pv")
                    nc.sync.dma_start(
                        out=pv_cur,
                        in_=PV_d[t:t + Tc].rearrange("t k p b -> p t k b"))
                ts_ = t % HB
                if ts_ == 0:
                    hbuf = houtp.tile([128, HB, KH, BLOC], F32, tag="hb")

                psA = psum.tile([128, 512], F32, tag="psA")
                ps_rv = psA[:, 0:GW].rearrange("p (k b) -> p k b", b=BLOC)
                ps_bT = psA[0:L, 32:32 + BLOC]
                ps_d = psA[0:1, 40:40 + BLOC]
                ps_hts = psA[:, 48:48 + GW].rearrange("p (k b) -> p k b", b=BLOC)
                ps_rbc = psA[:, 80:80 + BLOC]
                psB = psum.tile([128, 512], F32, tag="ghh")
                ps_ghh = psB[:, 0:KM * BLOC].rearrange("p (m b) -> p m b", b=BLOC)
                psC = psum.tile([128, 512], F32, tag="g2")
                ps_g2 = psC[:, 0:KM * BLOC]

                # --- PE: ident@PV preload + proj_R (chain head)
                for m in range(KH):
                    nc.tensor.matmul(
                        ps_rv[:, m, :], ident_sb,
                        pv_cur[:, ic, m, :], start=True, stop=False)
                    for kc in range(KH):
                        nc.tensor.matmul(
                            ps_rv[:, m, :],
                            wr_sb[:, kc, m * 128:(m + 1) * 128],
                            h_prev[:, kc * BLOC:(kc + 1) * BLOC],
                            start=False, stop=(kc == KH - 1))
                # --- PE: early gates partial: ident@GV + Whh @ Hc
                for m in range(KM):
                    nc.tensor.matmul(
                        ps_ghh[:, m, :], ident_sb, gv_cur[:, ic, m, :],
                        start=True, stop=False)
                    for kc in range(KH):
                        nc.tensor.matmul(
                            ps_ghh[:, m, :],
                            whh_sb[:, kc, m * 128:(m + 1) * 128],
                            h_prev[:, kc * BLOC:(kc + 1) * BLOC],
                            start=False, stop=(kc == KH - 1))
                ghh = work.tile([128, KM * BLOC], F32, tag="ghh_sb")
                nc.vector.tensor_copy(
                    ghh.rearrange("p (m b) -> p m b", b=BLOC), ps_ghh)

                # --- e = tanh(PS + bcast_L(ps_rv)): two PSUM-direct DVE
                #     chunks; chunk 2's add overlaps chunk 1's tanh
                e1 = work.tile([128, 2, BL], F32, tag="e1")
                e2 = work.tile([128, 2, BL], F32, tag="e2")
                for kp, et in ((0, e1), (1, e2)):
                    rv = psA[:, 16 * kp:16 * kp + 16]
                    bc = bass.AP(tensor=rv.tensor, offset=rv.offset,
                                 ap=[rv.ap[0], [BLOC, 2], [1, BLOC], [0, L]])
                    nc.vector.tensor_tensor(
                        et.rearrange("p k (b l) -> p k b l", l=L),
                        ps_sb[:, 2 * kp:2 * kp + 2].rearrange(
                            "p k (b l) -> p k b l", l=L),
                        bc, ALU.add)
                    nc.scalar.activation(
                        et.rearrange("p k n -> p (k n)"),
                        et.rearrange("p k n -> p (k n)"), AF.Tanh)

                # --- PE: beta_T [48, B] (e stationary, w moving), per-b cols
                for b in range(BLOC):
                    for kc in range(KH):
                        et = e1 if kc < 2 else e2
                        nc.tensor.matmul(
                            ps_bT[:, b:b + 1],
                            et[:, kc % 2, b * L:(b + 1) * L],
                            wvec_sb[:, kc:kc + 1],
                            start=(kc == 0), stop=(kc == KH - 1))
                # --- softmax: exp feeds h_ts matmuls directly; denominator
                #     via one ones-column matmul; 1/denom broadcast across
                #     partitions with one more matmul; one TT-mult normalize
                expT = work.tile([L, BLOC], F32, tag="expT")
                nc.scalar.activation(expT, ps_bT, AF.Exp, bias=bw_sb[0:L, 0:1])
                nc.tensor.matmul(ps_d, ones48_sb, expT, start=True, stop=True)
                for m in range(KH):
                    for b in range(BLOC):
                        nc.tensor.matmul(
                            ps_hts[:, m, b:b + 1],
                            hsbl_sb[:, b, m * 128:(m + 1) * 128],
                            expT[:, b:b + 1], start=True, stop=True)
                rec = work.tile([1, BLOC], F32, tag="rec")
                nc.vector.reciprocal(rec, ps_d)
                nc.tensor.matmul(ps_rbc, ones1_sb, rec, start=True, stop=True)
                htsr = work.tile([128, GW], F32, tag="htsr")
                nc.scalar.copy(htsr.rearrange("p (k b) -> p k b", b=BLOC), ps_hts)
                hts = work.tile([128, GW], F32, tag="hts")
                rbc_bc = bass.AP(tensor=psA.tensor, offset=ps_rbc.offset,
                                 ap=[ps_rbc.ap[0], [0, KH], [1, BLOC]])
                nc.vector.tensor_tensor(
                    hts.rearrange("p (k b) -> p k b", b=BLOC),
                    htsr.rearrange("p (k b) -> p k b", b=BLOC), rbc_bc, ALU.mult)

                # --- PE: late gates: ident@ghh + WihS @ h_ts
                for m in range(KM):
                    nc.tensor.matmul(
                        ps_g2[:, m * BLOC:(m + 1) * BLOC], ident_sb,
                        ghh[:, m * BLOC:(m + 1) * BLOC],
                        start=True, stop=False)
                    for kc in range(KH):
                        nc.tensor.matmul(
                            ps_g2[:, m * BLOC:(m + 1) * BLOC],
                            wihs_sb[:, kc, m * 128:(m + 1) * 128],
                            hts[:, kc * BLOC:(kc + 1) * BLOC],
                            start=False, stop=(kc == KH - 1))

                # --- LSTM pointwise (fused STT form; C = 2c, Hc = 2h)
                tg = work.tile([128, KM * BLOC], F32, tag="tgate")
                nc.scalar.activation(tg, ps_g2, AF.Tanh)
                ti, tf = tg[:, 0:GW], tg[:, GW:2 * GW]
                tgg, to = tg[:, 2 * GW:3 * GW], tg[:, 3 * GW:4 * GW]
                z1 = work.tile([128, GW], F32, tag="z1")
                nc.vector.scalar_tensor_tensor(
                    z1, tf, 1.0, c_prev, ALU.add, ALU.mult)  # (tf+1)*C
                z2 = work.tile([128, GW], F32, tag="z2")
                nc.vector.scalar_tensor_tensor(
                    z2, ti, 1.0, tgg, ALU.add, ALU.mult)     # (ti+1)*tg
                cn = state.tile([128, GW], F32, tag="c")
                nc.vector.scalar_tensor_tensor(
                    cn, z1, 0.5, z2, ALU.mult, ALU.add)      # C' = z1/2 + z2
                tc_ = work.tile([128, GW], F32, tag="tc")
                nc.scalar.activation(tc_, cn, AF.Tanh, scale=0.5)
                hn = state.tile([128, GW], F32, tag="h")
                nc.vector.scalar_tensor_tensor(
                    hn, to, 1.0, tc_, ALU.add, ALU.mult)     # Hc = (to+1)*tc
                nc.vector.tensor_scalar_mul(
                    hbuf[:, ts_], hn.rearrange("p (k b) -> p k b", b=BLOC), 0.5)

                c_prev, h_prev = cn, hn
                if ts_ == HB - 1 or t == T - 1:
                    nb = ts_ + 1
                    t0 = t - nb + 1
                    nc.sync.dma_start(
                        out=out_c[t0:t0 + nb].rearrange("t k p b -> p (t k) b"),
                        in_=hbuf[:, :nb].rearrange("p t k b -> p (t k) b"))
            psum.release()
    nc.finalize()
    return nc


# ---------------- host side ----------------

def prep_core_inputs(h_v, h_s, W, T=T_FULL):
    """Per-core input maps. W: dict of full weight arrays."""
    # i/f/o gate rows pre-scaled by 0.5 (tanh-form sigmoid); g rows kept.
    srow = np.concatenate([
        np.full(H, 0.5, np.float32), np.full(H, 0.5, np.float32),
        np.ones(H, np.float32), np.full(H, 0.5, np.float32)])
    W_ih = W["W_ih"] * srow[:, None]
    # recurrence carries Hc = 2h -> scale h-consuming weights by 0.5
    W_hh = W["W_hh"] * srow[:, None] * 0.5
    W_R = W["W_R"] * 0.5
    WS_T = np.ascontiguousarray(W["W_S"].T)
    WV_T = np.ascontiguousarray(W["W_V"].T)
    WihV_T = np.ascontiguousarray(W_ih[:, :DV].T)
    WihS_T = np.ascontiguousarray(W_ih[:, DV:].T)
    Whh_T = np.ascontiguousarray(W_hh.T)
    WR_T = np.ascontiguousarray(W_R.T)
    wvec = np.ascontiguousarray(W["W_w"][0][:, None])
    biasRSV = np.ascontiguousarray(
        (W["b_S"] + W["b_R"] + W["b_V"]).reshape(KH, 128).T)
    biasIH = np.ascontiguousarray(
        ((W["b_ih"] + W["b_hh"]) * srow).reshape(KM, 128).T)
    bw128 = np.full((128, 1), W["b_w"][0], np.float32)
    ident = np.eye(128, dtype=np.float32)
    maps = []
    for c in range(NCORES):
        bs = slice(c * BLOC, (c + 1) * BLOC)
        hvT = np.ascontiguousarray(
            h_v[bs, :T].transpose(2, 1, 0).reshape(DV, T * BLOC))
        hsT = np.ascontiguousarray(
            h_s[bs].transpose(2, 0, 1).reshape(DS, BLOC * L))
        hsblB = np.ascontiguousarray(h_s[bs])  # [B, L, D] -> [b, l, d]
        maps.append({
            "hvT": hvT, "hsT": hsT, "hsblB": hsblB, "WS_T": WS_T, "WV_T": WV_T,
            "WihV_T": WihV_T, "WihS_T": WihS_T, "Whh_T": Whh_T, "WR_T": WR_T,
            "wvec": wvec, "biasRSV": biasRSV, "biasIH": biasIH, "bw128": bw128,
            "ident": ident,
        })
    return maps


_NC_CACHE = {}


def kernel(**inputs):
    h_v = np.asarray(inputs["h_v"], dtype=np.float32)
    h_s = np.asarray(inputs["h_s"], dtype=np.float32)
    W = {k: np.asarray(v, dtype=np.float32) for k, v in inputs.items()}
    key = "full"
    if key not in _NC_CACHE:
        _NC_CACHE[key] = build_nc(T=T_FULL, Tc=16)
    nc = _NC_CACHE[key]
    maps = prep_core_inputs(h_v, h_s, W, T=T_FULL)
    res = run_bass_kernel_spmd(nc, maps, list(range(NCORES)))
    outs = []
    for c in range(NCORES):
        arr = res.results[c]["out_c"]  # [T, KH, 128, BLOC]
        outs.append(np.ascontiguousarray(
            arr.transpose(3, 0, 1, 2).reshape(BLOC, T_FULL, H)))
    return np.concatenate(outs, axis=0).astype(np.float32)


if __name__ == "__main__":
    nc = build_nc(T=8, Tc=4)
    print("built ok")


# revision 28
# speedup vs baseline: 1.0116x; 1.0116x over previous
"""Trainium2 Bass kernel for nn_Interactor (attention-augmented LSTM).

Problem: B=64, T=512, L=48, DV=DS=H=512.
  per step t: proj_V = x_t W_V^T; proj_R = h W_R^T
              e = tanh(proj_R[:,None,:] + proj_S + proj_V[:,None,:])
              alpha = softmax(e @ w, axis=L); h_ts = alpha @ h_s
              gates = [x_t, h_ts] W_ih^T + h W_hh^T + b; LSTM cell update.

Sharding: data-parallel over batch: 8 cores x 8 batch rows, weights replicated.

Per-core layout: feature dims on SBUF partitions, batch rows on the free dim.
x_t-dependent projections (PV = W_V x_t, GV = W_ihV x_t + b) are precomputed
with large matmuls into DRAM and streamed back per T-chunk.

The recurrent step is one serial dependency chain (wall time = T x chain
latency), so the design minimizes serial stages and cross-engine handoffs:
 - PV[t] is preloaded into the proj_R PSUM accumulator with an identity
   matmul; the e-add reads that PSUM directly with a stride-0 broadcast
   access pattern (no separate rvt stage). The e-add is split between
   VectorE (kc 0-1) and GpSimd/Pool (kc 2-3, from a small SBUF copy since
   Pool cannot read PSUM) into two separate tiles so the two tanh calls
   have independent dependencies.
 - All sigmoids are computed as tanh: host pre-scales i/f/o gate rows of
   W_ih/W_hh/b by 0.5, so ONE tanh over the gate tile yields tanh(i/2),
   tanh(f/2), tanh(g), tanh(o/2) (single activation-table set, no per-step
   reloads). The cell carries C = 2c and Hc = 2h; with fused
   scalar_tensor_tensor ops the cell update is 3 serial DVE stages:
     z1 = (tf+1)*C; z2 = (ti+1)*tg; C' = 0.5*z1 + z2
     tc = tanh(0.5*C'); Hc = (to+1)*tc; h_out = 0.5*Hc.
   (W_R/W_hh columns are pre-scaled by 0.5 to absorb Hc = 2h.)
 - beta = e @ w is computed transposed into a [48, B] per-batch-column
   PSUM layout (e stationary, w moving), so softmax needs NO masks:
   exp(beta) columns feed the h_ts matmuls directly (one [48,128] x
   [48,1] matmul per (d-chunk, b)), the denominator is one ones-column
   matmul, and 1/denom is broadcast across partitions with one more
   matmul; h_ts is normalized by a single tensor_tensor multiply.
 - gates accumulate in two PSUM passes: ident@GV + W_hh @ Hc early
   (overlapped with the attention front, copied to SBUF), then
   ident@that + W_ihS @ h_ts late, with tanh reading PSUM directly.
   All PSUM accumulation groups are strictly sequential per 2KB bank.
"""

import numpy as np

import concourse.bass as bass
import concourse.mybir as mybir
import concourse.bass_isa as bass_isa
import concourse.tile as tile
from concourse import bacc
from concourse.bass_utils import run_bass_kernel_spmd

F32 = mybir.dt.float32
BF16 = mybir.dt.bfloat16
AF = mybir.ActivationFunctionType
ALU = mybir.AluOpType
AX = mybir.AxisListType

B, T_FULL, L = 64, 512, 48
DV, DS, H = 512, 512, 512
G4 = 4 * H
NCORES = 8
BLOC = B // NCORES  # 8
BL = BLOC * L       # 384
KH = H // 128       # 4 H-chunks
KM = G4 // 128      # 16 gate-row chunks
GW = KH * BLOC      # 32


def build_nc(T=T_FULL, Tc=16):
    """Build the per-core Bass program (SPMD; same program all cores)."""
    assert T % Tc == 0
    nc = bacc.Bacc()

    # ---- DRAM I/O (per-core slices fed via in_maps) ----
    hvT = nc.declare_dram_parameter("hvT", [DV, T * BLOC], BF16, isOutput=False)
    hsT = nc.declare_dram_parameter("hsT", [DS, BL], F32, isOutput=False)
    hsblB = nc.declare_dram_parameter("hsblB", [BLOC, L, DS], F32, isOutput=False)
    WS_T = nc.declare_dram_parameter("WS_T", [DS, H], F32, isOutput=False)
    WV_T = nc.declare_dram_parameter("WV_T", [DV, H], BF16, isOutput=False)
    WihV_T = nc.declare_dram_parameter("WihV_T", [DV, G4], BF16, isOutput=False)
    WihS_T = nc.declare_dram_parameter("WihS_T", [DS, G4], F32, isOutput=False)
    Whh_T = nc.declare_dram_parameter("Whh_T", [H, G4], F32, isOutput=False)
    WR_T = nc.declare_dram_parameter("WR_T", [H, H], F32, isOutput=False)
    wvec = nc.declare_dram_parameter("wvec", [H, 1], F32, isOutput=False)
    biasRSV = nc.declare_dram_parameter("biasRSV", [128, KH], F32, isOutput=False)
    biasIH = nc.declare_dram_parameter("biasIH", [128, KM], F32, isOutput=False)
    bw128 = nc.declare_dram_parameter("bw128", [128, 1], F32, isOutput=False)
    ident = nc.declare_dram_parameter("ident", [128, 128], F32, isOutput=False)
    out_c = nc.declare_dram_parameter("out_c", [T, KH, 128, BLOC], F32, isOutput=True)

    # ---- internal DRAM for precomputed projections ----
    GV_d = nc.dram_tensor("GV_d", [T, KM, 128, BLOC], BF16)
    PV_d = nc.dram_tensor("PV_d", [T, KH, 128, BLOC], BF16)

    NT = T * BLOC  # hvT free size
    NCW = min(512, NT)  # precompute N-chunk width
    n_nc = NT // NCW

    with tile.TileContext(nc) as tc:
        with (
            tc.tile_pool(name="res", bufs=1) as res,       # resident weights etc
            tc.tile_pool(name="state", bufs=2) as state,   # c/h state ping-pong
            tc.tile_pool(name="hout", bufs=2) as houtp,    # h ring (8 steps)
            tc.tile_pool(name="stream", bufs=2) as stream, # GV/PV chunks
            tc.tile_pool(name="work", bufs=2) as work,     # per-step tiles
        ):
            # ---------- resident loads ----------
            wr_sb = res.tile([128, KH, H], F32, tag="wr")
            nc.sync.dma_start(out=wr_sb, in_=WR_T.rearrange("(k p) m -> p k m", p=128))
            wihs_sb = res.tile([128, KH, G4], F32, tag="wihs")
            nc.sync.dma_start(out=wihs_sb, in_=WihS_T.rearrange("(k p) m -> p k m", p=128))
            whh_sb = res.tile([128, KH, G4], F32, tag="whh")
            nc.sync.dma_start(out=whh_sb, in_=Whh_T.rearrange("(k p) m -> p k m", p=128))
            wvec_sb = res.tile([128, KH], F32, tag="wvec")
            nc.sync.dma_start(out=wvec_sb, in_=wvec.rearrange("(k p) o -> p (k o)", p=128))
            bw_sb = res.tile([128, 1], F32, tag="bw")
            nc.sync.dma_start(out=bw_sb, in_=bw128[:, :])
            ident_sb = res.tile([128, 128], F32, tag="ident")
            nc.sync.dma_start(out=ident_sb, in_=ident[:, :])
            identb_sb = res.tile([128, 128], BF16, tag="identb")
            nc.scalar.copy(identb_sb, ident_sb)
            ones48_sb = res.tile([L, 1], F32, tag="ones48")
            nc.vector.memset(ones48_sb, 1.0)
            ones1_sb = res.tile([1, 128], F32, tag="ones1")
            nc.vector.memset(ones1_sb, 1.0)
            # h_s in [l, b, d] layout: per (b): [48, 512] for the h_ts matmuls
            hsbl_sb = res.tile([L, BLOC, DS], F32, tag="hsbl")
            nc.sync.dma_start(out=hsbl_sb, in_=hsblB.rearrange("b p d -> p b d"))
            # PS: proj_S + (b_S+b_R+b_V), [128, KH, (b,l)]
            ps_sb = res.tile([128, KH, BL], F32, tag="ps")

            # ---------- precompute phase ----------
            with (
                tc.tile_pool(name="prew", bufs=1) as prew,
                tc.tile_pool(name="prehv", bufs=4) as prehv,
                tc.tile_pool(name="prestg", bufs=2) as prestg,
                tc.tile_pool(name="prepsum", bufs=4, space="PSUM") as prepsum,
            ):
                hs_sb = prew.tile([128, KH, BL], F32, tag="hs")
                nc.sync.dma_start(out=hs_sb, in_=hsT.rearrange("(k p) n -> p k n", p=128))
                brsv_sb = prew.tile([128, KH], F32, tag="brsv")
                nc.sync.dma_start(out=brsv_sb, in_=biasRSV[:, :])
                bih_sb = prew.tile([128, KM], F32, tag="bih")
                nc.sync.dma_start(out=bih_sb, in_=biasIH[:, :])
                ws_sb = prew.tile([128, KH, H], F32, tag="ws")
                nc.sync.dma_start(out=ws_sb, in_=WS_T.rearrange("(k p) m -> p k m", p=128))
                wv_sb = prew.tile([128, KH, H], BF16, tag="wv")
                nc.sync.dma_start(out=wv_sb, in_=WV_T.rearrange("(k p) m -> p k m", p=128))
                wihv_sb = prew.tile([128, KH, G4], BF16, tag="wihv")
                nc.sync.dma_start(out=wihv_sb, in_=WihV_T.rearrange("(k p) m -> p k m", p=128))

                # PS = W_S @ hsT + biasRSV
                for m in range(KH):
                    pps = prepsum.tile([128, 512], F32, tag="pp")
                    for kc in range(KH):
                        nc.tensor.matmul(
                            pps[:, :BL],
                            ws_sb[:, kc, m * 128:(m + 1) * 128],
                            hs_sb[:, kc, :],
                            start=(kc == 0), stop=(kc == KH - 1),
                        )
                    nc.vector.tensor_scalar_add(ps_sb[:, m, :], pps[:, :BL], brsv_sb[:, m:m + 1])

                # PV / GV over hvT N-chunks of 512
                for ncnk in range(n_nc):
                    nsl = slice(ncnk * NCW, (ncnk + 1) * NCW)
                    hv_t = []
                    for kc in range(KH):
                        t_ = prehv.tile([128, NCW], BF16, tag="hv")
                        nc.sync.dma_start(
                            out=t_, in_=hvT[kc * 128:(kc + 1) * 128, nsl])
                        hv_t.append(t_)
                    for m in range(KH):
                        ppv = prepsum.tile([128, NCW], F32, tag="pp")
                        for kc in range(KH):
                            nc.tensor.matmul(
                                ppv, wv_sb[:, kc, m * 128:(m + 1) * 128],
                                hv_t[kc], start=(kc == 0), stop=(kc == KH - 1))
                        stg = prestg.tile([128, NCW], BF16, tag="pvstg")
                        nc.scalar.copy(stg, ppv)
                        t0 = ncnk * NCW // BLOC
                        tw = NCW // BLOC
                        nc.sync.dma_start(
                            out=PV_d[t0:t0 + tw, m, :, :].rearrange("t p b -> p t b"),
                            in_=stg.rearrange("p (t b) -> p t b", b=BLOC))
                    for m in range(KM):
                        pgv = prepsum.tile([128, NCW], F32, tag="pp")
                        for kc in range(KH):
                            nc.tensor.matmul(
                                pgv, wihv_sb[:, kc, m * 128:(m + 1) * 128],
                                hv_t[kc], start=(kc == 0), stop=(kc == KH - 1))
                        stg = prestg.tile([128, NCW], BF16, tag="gvstg")
                        nc.vector.tensor_scalar_add(stg, pgv, bih_sb[:, m:m + 1])
                        t0 = ncnk * NCW // BLOC
                        tw = NCW // BLOC
                        nc.sync.dma_start(
                            out=GV_d[t0:t0 + tw, m, :, :].rearrange("t p b -> p t b"),
                            in_=stg.rearrange("p (t b) -> p t b", b=BLOC))

            # ---------- recurrence ----------
            psum = tc.alloc_tile_pool(name="psum", bufs=2, space="PSUM")
            czero = res.tile([128, GW], F32, tag="c0z")
            nc.vector.memset(czero, 0.0)
            hzero = res.tile([128, GW], F32, tag="h0z")
            nc.vector.memset(hzero, 0.0)
            c_prev = czero  # C = 2c, [128, (kc,b)]
            h_prev = hzero  # Hc = 2h

            HB = 8  # h ring steps per DMA
            gv_cur = pv_cur = None
            hbuf = None

            for t in range(T):
                ic = t % Tc
                if ic == 0:
                    gv_cur = stream.tile([128, Tc, KM, BLOC], BF16, tag="gv")
                    nc.sync.dma_start(
                        out=gv_cur,
                        in_=GV_d[t:t + Tc].rearrange("t m p b -> p t m b"))
                    pv_cur = stream.tile([128, Tc, KH, BLOC], BF16, tag="pv")
                    nc.sync.dma_start(
                        out=pv_cur,
                        in_=PV_d[t:t + Tc].rearrange("t k p b -> p t k b"))
                ts_ = t % HB
                if ts_ == 0:
                    hbuf = houtp.tile([128, HB, KH, BLOC], F32, tag="hb")

                psA = psum.tile([128, 512], F32, tag="psA")
                ps_rv = psA[:, 0:GW].rearrange("p (k b) -> p k b", b=BLOC)
                ps_bT = psA[0:L, 32:32 + BLOC]
                ps_d = psA[0:1, 40:40 + BLOC]
                ps_hts = psA[:, 48:48 + GW].rearrange("p (k b) -> p k b", b=BLOC)
                ps_rbc = psA[:, 80:80 + BLOC]
                psB = psum.tile([128, 512], F32, tag="ghh")
                ps_ghh = psB[:, 0:KM * BLOC].rearrange("p (m b) -> p m b", b=BLOC)
                psC = psum.tile([128, 512], F32, tag="g2")
                ps_g2 = psC[:, 0:KM * BLOC]

                # --- PE: ident@PV preload + proj_R (chain head)
                for m in range(KH):
                    nc.tensor.matmul(
                        ps_rv[:, m, :], identb_sb,
                        pv_cur[:, ic, m, :], start=True, stop=False)
                    for kc in range(KH):
                        nc.tensor.matmul(
                            ps_rv[:, m, :],
                            wr_sb[:, kc, m * 128:(m + 1) * 128],
                            h_prev[:, kc * BLOC:(kc + 1) * BLOC],
                            start=False, stop=(kc == KH - 1))
                # --- PE: early gates partial: ident@GV + Whh @ Hc
                for m in range(KM):
                    nc.tensor.matmul(
                        ps_ghh[:, m, :], identb_sb, gv_cur[:, ic, m, :],
                        start=True, stop=False)
                    for kc in range(KH):
                        nc.tensor.matmul(
                            ps_ghh[:, m, :],
                            whh_sb[:, kc, m * 128:(m + 1) * 128],
                            h_prev[:, kc * BLOC:(kc + 1) * BLOC],
                            start=False, stop=(kc == KH - 1))
                ghh = work.tile([128, KM * BLOC], F32, tag="ghh_sb")
                nc.vector.tensor_copy(
                    ghh.rearrange("p (m b) -> p m b", b=BLOC), ps_ghh)

                # --- e = tanh(PS + bcast_L(ps_rv)): two PSUM-direct DVE
                #     chunks; chunk 2's add overlaps chunk 1's tanh
                e1 = work.tile([128, 2, BL], F32, tag="e1")
                e2 = work.tile([128, 2, BL], F32, tag="e2")
                for kp, et in ((0, e1), (1, e2)):
                    rv = psA[:, 16 * kp:16 * kp + 16]
                    bc = bass.AP(tensor=rv.tensor, offset=rv.offset,
                                 ap=[rv.ap[0], [BLOC, 2], [1, BLOC], [0, L]])
                    nc.vector.tensor_tensor(
                        et.rearrange("p k (b l) -> p k b l", l=L),
                        ps_sb[:, 2 * kp:2 * kp + 2].rearrange(
                            "p k (b l) -> p k b l", l=L),
                        bc, ALU.add)
                    nc.scalar.activation(
                        et.rearrange("p k n -> p (k n)"),
                        et.rearrange("p k n -> p (k n)"), AF.Tanh)

                # --- PE: beta_T [48, B] (e stationary, w moving), per-b cols
                for b in range(BLOC):
                    for kc in range(KH):
                        et = e1 if kc < 2 else e2
                        nc.tensor.matmul(
                            ps_bT[:, b:b + 1],
                            et[:, kc % 2, b * L:(b + 1) * L],
                            wvec_sb[:, kc:kc + 1],
                            start=(kc == 0), stop=(kc == KH - 1))
                # --- softmax: exp feeds h_ts matmuls directly; denominator
                #     via one ones-column matmul; 1/denom broadcast across
                #     partitions with one more matmul; one TT-mult normalize
                expT = work.tile([L, BLOC], F32, tag="expT")
                nc.scalar.activation(expT, ps_bT, AF.Exp, bias=bw_sb[0:L, 0:1])
                nc.tensor.matmul(ps_d, ones48_sb, expT, start=True, stop=True)
                for m in range(KH):
                    for b in range(BLOC):
                        nc.tensor.matmul(
                            ps_hts[:, m, b:b + 1],
                            hsbl_sb[:, b, m * 128:(m + 1) * 128],
                            expT[:, b:b + 1], start=True, stop=True)
                rec = work.tile([1, BLOC], F32, tag="rec")
                nc.vector.reciprocal(rec, ps_d)
                nc.tensor.matmul(ps_rbc, ones1_sb, rec, start=True, stop=True)
                htsr = work.tile([128, GW], F32, tag="htsr")
                nc.scalar.copy(htsr.rearrange("p (k b) -> p k b", b=BLOC), ps_hts)
                hts = work.tile([128, GW], F32, tag="hts")
                rbc_bc = bass.AP(tensor=psA.tensor, offset=ps_rbc.offset,
                                 ap=[ps_rbc.ap[0], [0, KH], [1, BLOC]])
                nc.vector.tensor_tensor(
                    hts.rearrange("p (k b) -> p k b", b=BLOC),
                    htsr.rearrange("p (k b) -> p k b", b=BLOC), rbc_bc, ALU.mult)

                # --- PE: late gates: ident@ghh + WihS @ h_ts
                for m in range(KM):
                    nc.tensor.matmul(
                        ps_g2[:, m * BLOC:(m + 1) * BLOC], ident_sb,
                        ghh[:, m * BLOC:(m + 1) * BLOC],
                        start=True, stop=False)
                    for kc in range(KH):
                        nc.tensor.matmul(
                            ps_g2[:, m * BLOC:(m + 1) * BLOC],
                            wihs_sb[:, kc, m * 128:(m + 1) * 128],
                            hts[:, kc * BLOC:(kc + 1) * BLOC],
                            start=False, stop=(kc == KH - 1))

                # --- LSTM pointwise (fused STT form; C = 2c, Hc = 2h)
                tg = work.tile([128, KM * BLOC], F32, tag="tgate")
                nc.scalar.activation(tg, ps_g2, AF.Tanh)
                ti, tf = tg[:, 0:GW], tg[:, GW:2 * GW]
                tgg, to = tg[:, 2 * GW:3 * GW], tg[:, 3 * GW:4 * GW]
                z1 = work.tile([128, GW], F32, tag="z1")
                nc.vector.scalar_tensor_tensor(
                    z1, tf, 1.0, c_prev, ALU.add, ALU.mult)  # (tf+1)*C
                z2 = work.tile([128, GW], F32, tag="z2")
                nc.vector.scalar_tensor_tensor(
                    z2, ti, 1.0, tgg, ALU.add, ALU.mult)     # (ti+1)*tg
                cn = state.tile([128, GW], F32, tag="c")
                nc.vector.scalar_tensor_tensor(
                    cn, z1, 0.5, z2, ALU.mult, ALU.add)      # C' = z1/2 + z2
                tc_ = work.tile([128, GW], F32, tag="tc")
                nc.scalar.activation(tc_, cn, AF.Tanh, scale=0.5)
                hn = state.tile([128, GW], F32, tag="h")
                nc.vector.scalar_tensor_tensor(
                    hn, to, 1.0, tc_, ALU.add, ALU.mult)     # Hc = (to+1)*tc
                nc.vector.tensor_scalar_mul(
                    hbuf[:, ts_], hn.rearrange("p (k b) -> p k b", b=BLOC), 0.5)

                c_prev, h_prev = cn, hn
                if ts_ == HB - 1 or t == T - 1:
                    nb = ts_ + 1
                    t0 = t - nb + 1
                    nc.sync.dma_start(
                        out=out_c[t0:t0 + nb].rearrange("t k p b -> p (t k) b"),
                        in_=hbuf[:, :nb].rearrange("p t k b -> p (t k) b"))
            psum.release()
    nc.finalize()
    return nc


# ---------------- host side ----------------

def prep_core_inputs(h_v, h_s, W, T=T_FULL):
    """Per-core input maps. W: dict of full weight arrays."""
    # i/f/o gate rows pre-scaled by 0.5 (tanh-form sigmoid); g rows kept.
    srow = np.concatenate([
        np.full(H, 0.5, np.float32), np.full(H, 0.5, np.float32),
        np.ones(H, np.float32), np.full(H, 0.5, np.float32)])
    W_ih = W["W_ih"] * srow[:, None]
    # recurrence carries Hc = 2h -> scale h-consuming weights by 0.5
    W_hh = W["W_hh"] * srow[:, None] * 0.5
    W_R = W["W_R"] * 0.5
    WS_T = np.ascontiguousarray(W["W_S"].T)
    import ml_dtypes
    WV_T = np.ascontiguousarray(W["W_V"].T).astype(ml_dtypes.bfloat16)
    WihV_T = np.ascontiguousarray(W_ih[:, :DV].T).astype(ml_dtypes.bfloat16)
    WihS_T = np.ascontiguousarray(W_ih[:, DV:].T)
    Whh_T = np.ascontiguousarray(W_hh.T)
    WR_T = np.ascontiguousarray(W_R.T)
    wvec = np.ascontiguousarray(W["W_w"][0][:, None])
    biasRSV = np.ascontiguousarray(
        (W["b_S"] + W["b_R"] + W["b_V"]).reshape(KH, 128).T)
    biasIH = np.ascontiguousarray(
        ((W["b_ih"] + W["b_hh"]) * srow).reshape(KM, 128).T)
    bw128 = np.full((128, 1), W["b_w"][0], np.float32)
    ident = np.eye(128, dtype=np.float32)
    maps = []
    for c in range(NCORES):
        bs = slice(c * BLOC, (c + 1) * BLOC)
        hvT = np.ascontiguousarray(
            h_v[bs, :T].transpose(2, 1, 0).reshape(DV, T * BLOC)).astype(
                ml_dtypes.bfloat16)
        hsT = np.ascontiguousarray(
            h_s[bs].transpose(2, 0, 1).reshape(DS, BLOC * L))
        hsblB = np.ascontiguousarray(h_s[bs])  # [B, L, D] -> [b, l, d]
        maps.append({
            "hvT": hvT, "hsT": hsT, "hsblB": hsblB, "WS_T": WS_T, "WV_T": WV_T,
            "WihV_T": WihV_T, "WihS_T": WihS_T, "Whh_T": Whh_T, "WR_T": WR_T,
            "wvec": wvec, "biasRSV": biasRSV, "biasIH": biasIH, "bw128": bw128,
            "ident": ident,
        })
    return maps


_NC_CACHE = {}


def kernel(**inputs):
    h_v = np.asarray(inputs["h_v"], dtype=np.float32)
    h_s = np.asarray(inputs["h_s"], dtype=np.float32)
    W = {k: np.asarray(v, dtype=np.float32) for k, v in inputs.items()}
    key = "full"
    if key not in _NC_CACHE:
        _NC_CACHE[key] = build_nc(T=T_FULL, Tc=16)
    nc = _NC_CACHE[key]
    maps = prep_core_inputs(h_v, h_s, W, T=T_FULL)
    res = run_bass_kernel_spmd(nc, maps, list(range(NCORES)))
    outs = []
    for c in range(NCORES):
        arr = res.results[c]["out_c"]  # [T, KH, 128, BLOC]
        outs.append(np.ascontiguousarray(
            arr.transpose(3, 0, 1, 2).reshape(BLOC, T_FULL, H)))
    return np.concatenate(outs, axis=0).astype(np.float32)


if __name__ == "__main__":
    nc = build_nc(T=8, Tc=4)
    print("built ok")
